# revision 6
# baseline (speedup 1.0000x reference)
"""Trainium2 Bass kernel: transformer encoder layer (S=4096,B=2,D=512,H=8,F=2048),
causal attention + RoPE, distributed over 8 NeuronCores.

Sharding (SPMD: one program, per-core data):
  - LN1+RoPE: sequence-parallel (core c owns s in [512c, 512(c+1)), both batches)
  - AllGather(xr^T, xnorm^T)  [4.2MB/rank]
  - QKV + causal attention: head-parallel (core c owns head c, full S, both b)
  - AllToAll(attn_head^T)     [2MB/rank] -> each core gets all heads for its tokens
  - out_proj + residual + LN2 + FFN: token-parallel (core c owns its s-slice)
LayerNorm affine params are folded into downstream weights host-side.
Softmax denominators come free from a ones-column appended to V.
"""
import numpy as np
from contextlib import ExitStack

import concourse.bass as bass
import concourse.tile as tile
from concourse import bacc, mybir
from concourse.bass_utils import run_bass_kernel_spmd
from concourse.masks import make_identity

F32 = mybir.dt.float32
AF = mybir.ActivationFunctionType
ALU = mybir.AluOpType

S, B, D, H, Dh, F = 4096, 2, 512, 8, 64, 2048
W = 8                    # cores
SL = S // W              # 512 s-positions per core
TL = SL * B              # 1024 local tokens
EPS = 1e-5
SCALE = 1.0 / float(np.sqrt(Dh))  # 0.125

NT = TL // 128           # 8 local token tiles
NK = D // 128            # 4 contraction chunks over D
NF = F // 128            # 16 chunks over F
NS = S // 128            # 32 key tiles per batch

_NC_CACHE = {}
_GELU_OVERRIDE = None  # set to AF.Identity in sim tests (CoreSim lacks Gelu)


def _layer_norm_stats(nc, pool, x_t, eps_sb):
    """Returns (rstd [128,1], negmean_rstd [128,1]) for rows of x_t."""
    stats = pool.tile([128, 6], F32, tag="st")
    nc.vector.bn_stats(out=stats, in_=x_t)
    mv = pool.tile([128, 2], F32, tag="mv")
    nc.vector.bn_aggr(out=mv, in_=stats)
    sd = pool.tile([128, 1], F32, tag="sd")
    nc.scalar.activation(out=sd, in_=mv[:, 1:2], func=AF.Sqrt, bias=eps_sb)
    rstd = pool.tile([128, 1], F32, tag="rs")
    nc.vector.reciprocal(out=rstd, in_=sd)
    nm = pool.tile([128, 1], F32, tag="nm")
    nc.vector.tensor_mul(nm, mv[:, 0:1], rstd)
    nc.vector.tensor_scalar_mul(nm, nm, -1.0)
    return rstd, nm


def _build_nc(flags):
    """flags = (has_ropeb, has_bq, has_bk, has_bv, has_bo, has_b2)"""
    has_ropeb, has_bq, has_bk, has_bv, has_bo, has_b2 = flags
    nc = bacc.Bacc("TRN2", target_bir_lowering=False, debug=False, num_devices=W)

    # ---- I/O ----
    src_loc = nc.dram_tensor("src_loc", [TL, D], F32, kind="ExternalInput")
    cosw = nc.dram_tensor("cosw", [SL, D], F32, kind="ExternalInput")
    rotw = nc.dram_tensor("rotw", [SL, D], F32, kind="ExternalInput")
    ropeb = nc.dram_tensor("ropeb", [SL, D], F32, kind="ExternalInput") if has_ropeb else None
    wq_t = nc.dram_tensor("wq_t", [D, Dh], F32, kind="ExternalInput")
    wk_t = nc.dram_tensor("wk_t", [D, Dh], F32, kind="ExternalInput")
    wv_t = nc.dram_tensor("wv_t", [D, Dh], F32, kind="ExternalInput")
    bqkv = nc.dram_tensor("bqkv", [3, Dh], F32, kind="ExternalInput")
    wo_t = nc.dram_tensor("wo_t", [D, D], F32, kind="ExternalInput")
    bo = nc.dram_tensor("bo", [D], F32, kind="ExternalInput")
    w1_t = nc.dram_tensor("w1_t", [D, F], F32, kind="ExternalInput")
    b1p = nc.dram_tensor("b1p", [F], F32, kind="ExternalInput")
    w2_t = nc.dram_tensor("w2_t", [F, D], F32, kind="ExternalInput")
    b2 = nc.dram_tensor("b2", [D], F32, kind="ExternalInput")
    out_loc = nc.dram_tensor("out_loc", [TL, D], F32, kind="ExternalOutput")

    with tile.TileContext(nc) as tc, ExitStack() as top:
        dram = top.enter_context(tc.tile_pool(name="dram", bufs=1, space="DRAM"))
        consts = top.enter_context(tc.tile_pool(name="consts", bufs=1))
        persist = top.enter_context(tc.tile_pool(name="persist", bufs=1))

        # ---------- constants ----------
        ident = consts.tile([128, 128], F32)
        make_identity(nc, ident)
        # causal diag masks: masks[:, j, q] = 1.0 if q >= k + j*128 else 0.0
        masks = consts.tile([128, 4, 512], F32)
        for j in range(4):
            nc.gpsimd.memset(masks[:, j, :], 1.0)
            nc.gpsimd.affine_select(
                out=masks[:, j, :], in_=masks[:, j, :],
                compare_op=ALU.is_ge, fill=0.0,
                base=-j * 128, channel_multiplier=-1, pattern=[[1, 512]],
            )
        eps_sb = consts.tile([128, 1], F32)
        nc.vector.memset(eps_sb, EPS)
        bqkv_sb = consts.tile([Dh, 3], F32)
        nc.sync.dma_start(out=bqkv_sb, in_=bqkv.rearrange("o d -> d o"))
        bo_bc = consts.tile([128, D], F32)
        if has_bo:
            bo_row = consts.tile([1, D], F32)
            nc.sync.dma_start(out=bo_row, in_=bo[None, :])
            nc.gpsimd.partition_broadcast(bo_bc, bo_row)
        b2_bc = consts.tile([128, D], F32)
        if has_b2:
            b2_row = consts.tile([1, D], F32)
            nc.sync.dma_start(out=b2_row, in_=b2[None, :])
            nc.gpsimd.partition_broadcast(b2_bc, b2_row)
        b1_sb = consts.tile([128, NF], F32)
        nc.sync.dma_start(out=b1_sb, in_=b1p.rearrange("(m p) -> p m", p=128))

        # collective buffers
        cc1_in = dram.tile([2, NK, 128, TL], F32)
        cc1_out = dram.tile([W, 2, NK, 128, TL], F32, addr_space="Shared")
        cc2_in = dram.tile([W, Dh, B, SL], F32)
        cc2_out = dram.tile([W, Dh, B, SL], F32)

        # persistent across phases 5-6
        out1 = persist.tile([128, NT, D], F32)    # post-attention residual stream
        yT = persist.tile([128, NK, TL], F32)     # LN2 output, D-major

        # ================= P1: LN1 + RoPE + transpose =================
        with ExitStack() as ctx:
            sb = ctx.enter_context(tc.tile_pool(name="p1", bufs=3))
            small = ctx.enter_context(tc.tile_pool(name="p1s", bufs=4))
            trps = ctx.enter_context(tc.tile_pool(name="p1ps", bufs=4, space="PSUM"))
            for t in range(NT):
                s_t = sb.tile([128, D], F32, tag="s")
                nc.sync.dma_start(out=s_t, in_=src_loc[t * 128:(t + 1) * 128, :])
                rstd, nm = _layer_norm_stats(nc, small, s_t, eps_sb)
                xn_t = sb.tile([128, D], F32, tag="xn")
                nc.vector.tensor_scalar(
                    out=xn_t, in0=s_t, scalar1=rstd, scalar2=nm,
                    op0=ALU.mult, op1=ALU.add,
                )
                # RoPE (ln1 affine folded into cosw/rotw/ropeb host-side)
                sc = t % (SL // 128)
                cosw_t = sb.tile([128, D], F32, tag="cw")
                nc.sync.dma_start(out=cosw_t, in_=cosw[sc * 128:(sc + 1) * 128, :])
                rotw_t = sb.tile([128, D], F32, tag="rw")
                nc.sync.dma_start(out=rotw_t, in_=rotw[sc * 128:(sc + 1) * 128, :])
                xr_t = sb.tile([128, D], F32, tag="xr")
                rt = sb.tile([128, D], F32, tag="rt")
                xnv = xn_t.rearrange("p (h i two) -> p h i two", h=H, two=2)
                rtv = rt.rearrange("p (h d) -> p h d", h=H)
                rwv = rotw_t.rearrange("p (h d) -> p h d", h=H)
                # rt[:, :, :32] = xn[:, :, 1::2] * rotw[:, :, :32]
                nc.vector.tensor_mul(rtv[:, :, 0:32], xnv[:, :, :, 1], rwv[:, :, 0:32])
                # rt[:, :, 32:] = xn[:, :, 0::2] * rotw[:, :, 32:]
                nc.vector.tensor_mul(rtv[:, :, 32:64], xnv[:, :, :, 0], rwv[:, :, 32:64])
                nc.vector.tensor_mul(xr_t, xn_t, cosw_t)
                nc.vector.tensor_add(xr_t, xr_t, rt)
                if has_ropeb:
                    rb_t = sb.tile([128, D], F32, tag="rb")
                    nc.sync.dma_start(out=rb_t, in_=ropeb[sc * 128:(sc + 1) * 128, :])
                    nc.vector.tensor_add(xr_t, xr_t, rb_t)
                # transpose to D-major and ship to the collective input buffer
                for idx, src_tile in ((0, xr_t), (1, xn_t)):
                    ps = trps.tile([128, 512], F32, tag="tr")
                    for k in range(NK):
                        nc.tensor.transpose(ps[:, k * 128:(k + 1) * 128],
                                            src_tile[:, k * 128:(k + 1) * 128], ident)
                    tmp = sb.tile([128, NK, 128], F32, tag="tmp")
                    nc.vector.tensor_copy(tmp, ps.rearrange("p (k i) -> p k i", k=NK))
                    nc.sync.dma_start(
                        out=cc1_in[idx][:, :, t * 128:(t + 1) * 128].rearrange(
                            "k p i -> p k i"),
                        in_=tmp)

        nc.gpsimd.collective_compute(
            "AllGather", ALU.bypass,
            ins=[cc1_in.opt()], outs=[cc1_out.opt()],
            replica_groups=[list(range(W))],
        )

        with ExitStack() as actx:
            act = actx.enter_context(tc.tile_pool(name="act", bufs=1))
            qT = act.tile([Dh, B, S], F32)
            kT = act.tile([Dh, B, S], F32)
            vS = act.tile([128, B, NS, 65], F32)
            nc.vector.memset(vS[:, :, :, 64:65], 1.0)
            attnT = act.tile([Dh, B, S], F32)

            # ============ P2: QKV projections (head h = this core) ============
            with ExitStack() as ctx:
                sb = ctx.enter_context(tc.tile_pool(name="p2", bufs=2))
                wpool = ctx.enter_context(tc.tile_pool(name="p2w", bufs=1))
                qkps = ctx.enter_context(tc.tile_pool(name="p2ps", bufs=2, space="PSUM"))
                vtps = ctx.enter_context(tc.tile_pool(name="p2vt", bufs=2, space="PSUM"))
                wq_sb = wpool.tile([128, NK, Dh], F32)
                nc.sync.dma_start(out=wq_sb, in_=wq_t.rearrange("(k p) m -> p k m", p=128))
                wk_sb = wpool.tile([128, NK, Dh], F32)
                nc.sync.dma_start(out=wk_sb, in_=wk_t.rearrange("(k p) m -> p k m", p=128))
                wv_sb = wpool.tile([128, NK, Dh], F32)
                nc.sync.dma_start(out=wv_sb, in_=wv_t.rearrange("(k p) m -> p k m", p=128))
                for j in range(W):
                    for b in range(B):
                        tok0 = b * SL
                        scol = j * SL
                        xr_in = sb.tile([128, NK, SL], F32, tag="xrin")
                        nc.sync.dma_start(
                            out=xr_in,
                            in_=cc1_out[j, 0].rearrange("k p t -> p k t")[:, :, tok0:tok0 + SL])
                        xn_in = sb.tile([128, NK, SL], F32, tag="xnin")
                        nc.sync.dma_start(
                            out=xn_in,
                            in_=cc1_out[j, 1].rearrange("k p t -> p k t")[:, :, tok0:tok0 + SL])
                        # q^T, k^T  [64, 512] blocks
                        for (wsb, dstT, hasb, bi) in ((wq_sb, qT, has_bq, 0),
                                                      (wk_sb, kT, has_bk, 1)):
                            ps = qkps.tile([Dh, SL], F32, tag="qk")
                            for k in range(NK):
                                nc.tensor.matmul(ps, wsb[:, k, :], xr_in[:, k, :],
                                                 start=(k == 0), stop=(k == NK - 1))
                            if hasb:
                                nc.vector.tensor_scalar_add(
                                    dstT[:, b, scol:scol + SL], ps, bqkv_sb[:, bi:bi + 1])
                            else:
                                nc.vector.tensor_copy(dstT[:, b, scol:scol + SL], ps)
                        # v -> token-major via PE transpose
                        ps = qkps.tile([Dh, SL], F32, tag="v")
                        for k in range(NK):
                            nc.tensor.matmul(ps, wv_sb[:, k, :], xn_in[:, k, :],
                                             start=(k == 0), stop=(k == NK - 1))
                        vtmp = sb.tile([Dh, SL], F32, tag="vtmp")
                        if has_bv:
                            nc.vector.tensor_scalar_add(vtmp, ps, bqkv_sb[:, 2:3])
                        else:
                            nc.vector.tensor_copy(vtmp, ps)
                        vt = vtps.tile([128, 4, Dh], F32, tag="vt")
                        for q4 in range(4):
                            nc.tensor.transpose(vt[:, q4, :],
                                                vtmp[:, q4 * 128:(q4 + 1) * 128],
                                                ident[0:Dh, 0:Dh])
                        nc.vector.tensor_copy(vS[:, b, j * 4:(j + 1) * 4, 0:64], vt)

            # ============ P4: causal attention for this head ============
            with ExitStack() as ctx:
                expp = ctx.enter_context(tc.tile_pool(name="p4e", bufs=4))
                nrm = ctx.enter_context(tc.tile_pool(name="p4n", bufs=3))
                scps = ctx.enter_context(tc.tile_pool(name="p4s", bufs=2, space="PSUM"))
                atps = ctx.enter_context(tc.tile_pool(name="p4a", bufs=2, space="PSUM"))
                for b in range(B):
                    for qb in range(8):
                        nkt = 4 * (qb + 1)
                        q_rhs = qT[:, b, qb * 512:(qb + 1) * 512]
                        pa = atps.tile([65, 512], F32, tag="pa")
                        for kt2 in range(nkt // 2):
                            ps = scps.tile([128, 1024], F32, tag="sc")
                            for i in range(2):
                                kt = kt2 * 2 + i
                                nc.tensor.matmul(ps[:, i * 512:(i + 1) * 512],
                                                 kT[:, b, kt * 128:(kt + 1) * 128],
                                                 q_rhs, start=True, stop=True)
                            ex = expp.tile([128, 1024], F32, tag="ex")
                            nc.scalar.activation(out=ex, in_=ps, func=AF.Exp, scale=SCALE)
                            for i in range(2):
                                kt = kt2 * 2 + i
                                j = kt - (nkt - 4)
                                if j >= 0:
                                    nc.vector.tensor_mul(ex[:, i * 512:(i + 1) * 512],
                                                         ex[:, i * 512:(i + 1) * 512],
                                                         masks[:, j, :])
                                nc.tensor.matmul(pa, vS[:, b, kt, :],
                                                 ex[:, i * 512:(i + 1) * 512],
                                                 start=(kt == 0), stop=(kt == nkt - 1))
                        pa_sb = nrm.tile([65, 512], F32, tag="pasb")
                        nc.vector.tensor_copy(pa_sb, pa)
                        sums = nrm.tile([1, 512], F32, tag="sums")
                        nc.sync.dma_start(out=sums, in_=pa_sb[64:65, :])
                        rcp = nrm.tile([1, 512], F32, tag="rcp")
                        nc.vector.reciprocal(rcp, sums)
                        rcp_bc = nrm.tile([Dh, 512], F32, tag="rbc")
                        nc.gpsimd.partition_broadcast(rcp_bc, rcp)
                        nc.vector.tensor_mul(attnT[:, b, qb * 512:(qb + 1) * 512],
                                             pa_sb[0:64, :], rcp_bc)
                nc.sync.dma_start(
                    out=cc2_in.rearrange("j d b i -> d b j i"),
                    in_=attnT.rearrange("d b (j i) -> d b j i", j=W))

        nc.gpsimd.collective_compute(
            "AllToAll", ALU.bypass,
            ins=[cc2_in.opt()], outs=[cc2_out.opt()],
            replica_groups=[list(range(W))],
        )

        # ========== P5: out_proj + residual + LN2 (+ transpose y) ==========
        with ExitStack() as ctx:
            sb = ctx.enter_context(tc.tile_pool(name="p5", bufs=3))
            small = ctx.enter_context(tc.tile_pool(name="p5s", bufs=4))
            wpool = ctx.enter_context(tc.tile_pool(name="p5w", bufs=1))
            ops = ctx.enter_context(tc.tile_pool(name="p5ps", bufs=2, space="PSUM"))
            trps = ctx.enter_context(tc.tile_pool(name="p5tr", bufs=2, space="PSUM"))
            wo_sb = wpool.tile([128, NK, D], F32)
            nc.sync.dma_start(out=wo_sb, in_=wo_t.rearrange("(k p) n -> p k n", p=128))
            for t in range(NT):
                b, sc = t // (NT // B), t % (NT // B)
                po = ops.tile([128, D], F32, tag="po")
                for k in range(NK):
                    a_sb = sb.tile([128, 128], F32, tag="a")
                    nc.sync.dma_start(
                        out=a_sb,
                        in_=cc2_out[2 * k:2 * k + 2, :, b,
                                    sc * 128:(sc + 1) * 128].rearrange("e d i -> (e d) i"))
                    nc.tensor.matmul(po, a_sb, wo_sb[:, k, :],
                                     start=(k == 0), stop=(k == NK - 1))
                s_t = sb.tile([128, D], F32, tag="s")
                nc.sync.dma_start(out=s_t, in_=src_loc[t * 128:(t + 1) * 128, :])
                o1 = out1[:, t, :]
                nc.vector.tensor_add(o1, po, s_t)
                if has_bo:
                    nc.vector.tensor_add(o1, o1, bo_bc)
                # LN2 (affine folded into w1_t/b1p host-side)
                rstd, nm = _layer_norm_stats(nc, small, o1, eps_sb)
                y_t = sb.tile([128, D], F32, tag="y")
                nc.vector.tensor_scalar(out=y_t, in0=o1, scalar1=rstd, scalar2=nm,
                                        op0=ALU.mult, op1=ALU.add)
                ps = trps.tile([128, 512], F32, tag="tr")
                for k in range(NK):
                    nc.tensor.transpose(ps[:, k * 128:(k + 1) * 128],
                                        y_t[:, k * 128:(k + 1) * 128], ident)
                nc.vector.tensor_copy(
                    yT[:, :, t * 128:(t + 1) * 128],
                    ps.rearrange("p (k i) -> p k i", k=NK))

        # ================= P6: FFN + final residual =================
        with ExitStack() as ctx:
            sb = ctx.enter_context(tc.tile_pool(name="p6", bufs=3))
            wpool = ctx.enter_context(tc.tile_pool(name="p6w", bufs=1))
            hps = ctx.enter_context(tc.tile_pool(name="p6h", bufs=2, space="PSUM"))
            o2ps = ctx.enter_context(tc.tile_pool(name="p6o", bufs=1, space="PSUM"))
            w1_sb = wpool.tile([128, NK, F], F32)
            nc.sync.dma_start(out=w1_sb, in_=w1_t.rearrange("(k p) n -> p k n", p=128))
            w2_sb = wpool.tile([128, NF, D], F32)
            nc.sync.dma_start(out=w2_sb, in_=w2_t.rearrange("(m p) n -> p m n", p=128))
            for th in range(2):
                po2 = [o2ps.tile([128, D], F32, tag=f"po2_{tq}", name=f"po2_{tq}")
                       for tq in range(4)]
                for m in range(NF):
                    ph = hps.tile([128, 512], F32, tag="ph")
                    for k in range(NK):
                        nc.tensor.matmul(ph, w1_sb[:, k, m * 128:(m + 1) * 128],
                                         yT[:, k, th * 512:(th + 1) * 512],
                                         start=(k == 0), stop=(k == NK - 1))
                    hT = sb.tile([128, 512], F32, tag="hT")
                    nc.scalar.activation(out=hT, in_=ph,
                                         func=_GELU_OVERRIDE or AF.Gelu,
                                         bias=b1_sb[:, m:m + 1])
                    for tq in range(4):
                        nc.tensor.matmul(po2[tq], hT[:, tq * 128:(tq + 1) * 128],
                                         w2_sb[:, m, :],
                                         start=(m == 0), stop=(m == NF - 1))
                for tq in range(4):
                    t = th * 4 + tq
                    fin = sb.tile([128, D], F32, tag="fin")
                    nc.vector.tensor_add(fin, po2[tq], out1[:, t, :])
                    if has_b2:
                        nc.vector.tensor_add(fin, fin, b2_bc)
                    nc.sync.dma_start(out=out_loc[t * 128:(t + 1) * 128, :], in_=fin)

    nc.compile()
    return nc


def _prep(inputs):
    src = np.asarray(inputs["src"], np.float32)
    cos = np.asarray(inputs["rotary_cos"], np.float32).reshape(S, Dh)
    sin = np.asarray(inputs["rotary_sin"], np.float32).reshape(S, Dh)
    ipw = np.asarray(inputs["in_proj_w"], np.float32)
    ipb = np.asarray(inputs["in_proj_b"], np.float32)
    opw = np.asarray(inputs["out_proj_w"], np.float32)
    opb = np.asarray(inputs["out_proj_b"], np.float32)
    w1 = np.asarray(inputs["w1"], np.float32)
    b1 = np.asarray(inputs["b1"], np.float32)
    w2 = np.asarray(inputs["w2"], np.float32)
    b2 = np.asarray(inputs["b2"], np.float32)
    ln1_w = np.asarray(inputs["ln1_w"], np.float32)
    ln1_b = np.asarray(inputs["ln1_b"], np.float32)
    ln2_w = np.asarray(inputs["ln2_w"], np.float32)
    ln2_b = np.asarray(inputs["ln2_b"], np.float32)

    cos_full = np.tile(cos, (1, H))            # [S, D]
    sin_full = np.tile(sin, (1, H))
    d = np.arange(D)
    jj = d % Dh
    hb = d - jj
    src2 = np.where(jj < 32, hb + 2 * jj + 1, hb + 2 * (jj - 32))
    sign = np.where(jj < 32, -1.0, 1.0).astype(np.float32)
    cosw_full = ln1_w[None, :] * cos_full
    rotw_full = (sign[None, :] * ln1_w[src2][None, :]) * sin_full
    ropeb_full = (ln1_b[None, :] * cos_full
                  + (sign[None, :] * ln1_b[src2][None, :]) * sin_full)

    wq, wk, wv = ipw[0:D], ipw[D:2 * D], ipw[2 * D:3 * D]
    bq, bk, bv = ipb[0:D], ipb[D:2 * D], ipb[2 * D:3 * D]
    w1_t = np.ascontiguousarray(ln2_w[:, None] * w1.T, np.float32)   # [D, F]
    b1p = np.ascontiguousarray(ln2_b @ w1.T + b1, np.float32)
    wo_t = np.ascontiguousarray(opw.T)

    flags = (
        bool(np.any(ropeb_full)), bool(np.any(bq)), bool(np.any(bk)),
        bool(np.any(bv) or np.any(ln1_b)), bool(np.any(opb)), bool(np.any(b2)),
    )

    in_maps = []
    for c in range(W):
        h0 = c * Dh
        wv_h = wv[h0:h0 + Dh]                                        # [64, D]
        wv_t_c = np.ascontiguousarray(ln1_w[:, None] * wv_h.T, np.float32)
        bv_c = np.ascontiguousarray(ln1_b @ wv_h.T + bv[h0:h0 + Dh], np.float32)
        m = {
            "src_loc": np.ascontiguousarray(
                src[SL * c:SL * (c + 1)].transpose(1, 0, 2).reshape(TL, D)),
            "cosw": np.ascontiguousarray(cosw_full[SL * c:SL * (c + 1)]),
            "rotw": np.ascontiguousarray(rotw_full[SL * c:SL * (c + 1)]),
            "wq_t": np.ascontiguousarray(wq[h0:h0 + Dh].T),
            "wk_t": np.ascontiguousarray(wk[h0:h0 + Dh].T),
            "wv_t": wv_t_c,
            "bqkv": np.stack([bq[h0:h0 + Dh], bk[h0:h0 + Dh], bv_c]),
            "wo_t": wo_t,
            "bo": opb,
            "w1_t": w1_t,
            "b1p": b1p,
            "w2_t": np.ascontiguousarray(w2.T),
            "b2": b2,
        }
        if flags[0]:
            m["ropeb"] = np.ascontiguousarray(ropeb_full[SL * c:SL * (c + 1)])
        in_maps.append(m)
    return in_maps, flags


def _get_nc(flags):
    if flags not in _NC_CACHE:
        _NC_CACHE[flags] = _build_nc(flags)
    return _NC_CACHE[flags]


def kernel(**inputs):
    in_maps, flags = _prep(inputs)
    nc = _get_nc(flags)
    res = run_bass_kernel_spmd(nc, in_maps, core_ids=list(range(W)))
    out = np.empty((S, B, D), np.float32)
    for c in range(W):
        ol = res.results[c]["out_loc"].reshape(B, SL, D)
        out[SL * c:SL * (c + 1)] = ol.transpose(1, 0, 2)
    return out
